# revision 30
# baseline (speedup 1.0000x reference)
"""Trainium2 Bass kernel for nn_DetectorKmeans (retrieval_knn).

density[n] = sum_k (pr[k]*var[k]) / ||X[n]-C[k]||^2  - threshold

Data-parallel over 8 NeuronCores (X sharded along N). Per core, per
"unit" = (256-row half-supertile, full K):

  * The per-column weight w_k is FOLDED INTO THE MATMUL via a per-column
    scale s_k: PSUM T[:,k] = s_k * sqdist. For the 768 largest-w columns
    s_k = 1/w_k, so ACT's Reciprocal emits w_k/sqdist directly and its
    free-dim accum_out produces the weighted k-sum AT FP32 FOR FREE.
    The 256 smallest-w columns (where 1/w_k would overflow the fp8 cm)
    use s_k = 16 and are host-permuted to k-positions 768..1023; a DVE
    reciprocal + narrow scalar_tensor_tensor (x 16*w_k) covers them.
    This removes the full-width DVE reduce (was 2 x 1024-elem STT at a
    fixed 2 cycles/elem = the kernel-wide bottleneck tail).
  * PSUM buffer [128, 2, 1024] (4 banks; pool bufs=2 = all 8 banks).
  * 5-row augmented matmuls run in disjoint 32-row PE groups
    (tile_position=(32t,0)), adding s_k*xsq[n] (3-term bf16 hi/lo
    product) and s_k*csq[k] (2-term) for the 4 row-tiles; then fp8
    DoubleRow mains (2 contraction chunks of 256) accumulate the cross
    term at 2x bf16 streaming rate.
  * DMA queues: sync = cq (aug const, host-replicated 4x) + xt stream +
    output stores; scalar = ACT table load, then cm (h=0 halves first,
    matching the h-outer main order) + wk_small.
"""

import numpy as np
import ml_dtypes

BF16 = ml_dtypes.bfloat16

N, K, D = 65536, 1024, 512
NCORES = 8
R = N // NCORES
F = 512  # rows per supertile
KH = 512  # k-half (PSUM bank width in fp32)
NSUP = R // F
KS = 128  # small-w columns handled by DVE (host-permuted to the tail)
KB = K - KS
S_SMALL = 16.0  # power of two: exact fp8 exponent shift
AUGN = 5

_NC = None


def _act_recip(nc, mybir, out, in_, accum_out=None):
    """ACT-engine reciprocal (bypasses the library guard; measured max rel
    err ~1.2e-5 on TRN2 HW for this kernel's value range). With accum_out
    the engine also emits the free-dim sum at fp32 -- the weighted reduce
    comes for free because w is pre-folded into the PSUM column scale."""
    dt = mybir.dt
    eng = nc.scalar
    ins = [
        eng.lower_ap(in_),
        mybir.ImmediateValue(dtype=dt.float32, value=0.0),
        mybir.ImmediateValue(dtype=dt.float32, value=1.0),
        mybir.ImmediateValue(dtype=dt.float32, value=0.0),
    ]
    outs = [eng.lower_ap(out)]
    if accum_out is not None:
        outs.append(eng.lower_ap(accum_out))
    return eng.add_instruction(
        mybir.InstActivation(
            name=nc.get_next_instruction_name(),
            func=mybir.ActivationFunctionType.Reciprocal,
            ins=ins,
            outs=outs,
        )
    )


def _tt_add(nc, mybir, out, in0, in1):
    """DVE tensor_tensor add (no bass builder exists)."""
    eng = nc.vector
    return eng.add_instruction(
        mybir.InstTensorTensor(
            name=nc.get_next_instruction_name(),
            op=mybir.AluOpType.add,
            ins=[eng.lower_ap(in0), eng.lower_ap(in1)],
            outs=[eng.lower_ap(out)],
        )
    )


def _build_nc(r=R, num_devices=NCORES):
    import concourse.bacc as bacc
    import concourse.tile as tile
    import concourse.mybir as mybir

    import os

    dt = mybir.dt
    nsup = r // F
    cqw = 2 * KH + r
    nc = bacc.Bacc(
        "TRN2", target_bir_lowering=False, debug=False, num_devices=num_devices
    )
    _salt = os.environ.get("KERNEL_SALT", "")
    xt_d = nc.dram_tensor("xt", [2, 128, 2, r], dt.float8e4, kind="ExternalInput")
    cm_d = nc.dram_tensor("cm", [2, 128, 2, K], dt.float8e4, kind="ExternalInput")
    wk_d = nc.dram_tensor("wk", [1, KS], dt.bfloat16, kind="ExternalInput")
    cq_d = nc.dram_tensor("cq", [4, AUGN, cqw], dt.bfloat16, kind="ExternalInput")
    out_d = nc.dram_tensor("out", [r], dt.float32, kind="ExternalOutput")

    with tile.TileContext(nc) as tc:
        with (
            tc.tile_pool(name="const" + _salt, bufs=1) as constp,
            tc.tile_pool(name="xin", bufs=4) as xinp,
            tc.tile_pool(name="rec", bufs=4) as recp,
            tc.tile_pool(name="osb", bufs=4) as osbp,
            tc.tile_pool(name="psT", bufs=2, space="PSUM") as psT,
        ):
            # cq groups 0/1 on sync, 2/3 on scalar -- two queues drain the
            # 4 small triggers in parallel; host replicated the rows 4x.
            cq = constp.tile([128, cqw], dt.bfloat16)
            for g in range(2):
                nc.sync.dma_start(cq[32 * g : 32 * g + AUGN, :], cq_d[g])
            for g in range(2, 4):
                nc.scalar.dma_start(cq[32 * g : 32 * g + AUGN, :], cq_d[g])
            carq = cq[:, : 2 * KH].rearrange("p (h k) -> p h k", h=2)
            auga = cq[:, 2 * KH :]
            # cm on the scalar queue, h=0 halves first (matches h-outer
            # main order so unit 0 h=0 can start earliest).
            cm = constp.tile([128, 2, 2, K], dt.float8e4)
            cm_r = cm_d.rearrange("c p e k -> p c e k")
            for h in range(2):
                for c in range(2):
                    nc.scalar.dma_start(
                        cm[:, c, :, KH * h : KH * (h + 1)],
                        cm_r[:, c, :, KH * h : KH * (h + 1)],
                    )
            wks = constp.tile([128, KS], dt.bfloat16)
            nc.scalar.dma_start(wks[:], wk_d[:].partition_broadcast(128))
            xt_r = xt_d.rearrange("c p e n -> p c e n")

            pending_store = None
            for s in range(nsup):
                n0 = s * F
                xt = xinp.tile([128, 2, 2, F], dt.float8e4, tag="xt")
                for c in range(2):
                    nc.sync.dma_start(xt[:, c, :, :], xt_r[:, c, :, n0 : n0 + F])
                if s % 4 == 0:
                    osbA = osbp.tile([128, 16], dt.float32, tag="osbA")
                    osbB = osbp.tile([128, 16], dt.float32, tag="osbB")

                def augs(T, u):
                    # all four aug matmuls in disjoint row groups -> one
                    # concurrent span; every partition group holds ALL arx
                    # columns, so the h=1 augs just read group g's columns
                    # from row group g+2.
                    for h in range(2):
                        for tl in range(2):
                            g = 2 * u + tl
                            gp = g if h == 0 else (g + 2) % 4
                            a0 = n0 + 128 * g
                            nc.tensor.matmul(
                                T[:, tl, KH * h : KH * (h + 1)],
                                auga[32 * gp : 32 * gp + AUGN, a0 : a0 + 128],
                                carq[32 * gp : 32 * gp + AUGN, h, :],
                                start=True,
                                stop=False,
                                tile_position=(32 * gp, 0),
                            )

                def mains(T, u):
                    # h-outer so the first unit only waits on the h=0 half
                    # of cm; accumulation group per (tl, h) stays c0->c1.
                    for h in range(2):
                        for tl in range(2):
                            g = 2 * u + tl
                            for c in range(2):
                                nc.tensor.matmul(
                                    T[:, tl, KH * h : KH * (h + 1)],
                                    xt[:, c, :, 128 * g : 128 * (g + 1)],
                                    cm[:, c, :, KH * h : KH * (h + 1)],
                                    perf_mode=mybir.MatmulPerfMode.DoubleRow,
                                    start=False,
                                    stop=(c == 1),
                                )

                def post(T, u):
                    # w is folded into the PSUM column scale, so the row
                    # density is a PLAIN sum of the reciprocal dump.
                    # Balance that reduce across engines: 1/4 of units use
                    # ACT's free-dim accum (costs +130ns instr split +
                    # 288ns accumulator read each), the rest use 1-source
                    # DVE tensor_scalar sums (single read port -> the
                    # accumulator readback port stays free, full rate).
                    # Small tail columns (weighted 1/16) get a narrow STT
                    # correction of sum (16 w_k - 1) * rr either way.
                    scr = recp.tile([128, 2, KS], dt.bfloat16, tag="scr")
                    dump = recp.tile([128, 2, K], dt.bfloat16, tag="dump")
                    act_mode = (2 * s + u) % 8 in (0, 3, 5)
                    if act_mode:
                        for tl in range(2):
                            col = 4 * (s % 4) + 2 * u + tl
                            _act_recip(
                                nc,
                                mybir,
                                dump[:, tl, :],
                                T[:, tl, :],
                                accum_out=osbA[:, col : col + 1],
                            )
                    else:
                        _act_recip(nc, mybir, dump[:], T[:])
                        for tl in range(2):
                            col = 4 * (s % 4) + 2 * u + tl
                            nc.vector.tensor_reduce(
                                osbA[:, col : col + 1],
                                dump[:, tl, :],
                                axis=mybir.AxisListType.X,
                                op=mybir.AluOpType.add,
                            )
                    for tl in range(2):
                        col = 4 * (s % 4) + 2 * u + tl
                        nc.vector.scalar_tensor_tensor(
                            scr[:, tl, :],
                            dump[:, tl, KB:],
                            0.0,
                            wks[:],
                            op0=mybir.AluOpType.bypass,
                            op1=mybir.AluOpType.mult,
                            accum_out=osbB[:, col : col + 1],
                        )

                if s == 0:
                    # pipeline fill: both units' augs run as soon as cq
                    # lands (each aug's weight load waits only on its own
                    # cq group's DMA -- verified minimal in the BIR).
                    T0 = psT.tile([128, 2, K], dt.float32, tag="T", name="T0")
                    T1 = psT.tile([128, 2, K], dt.float32, tag="T", name="T1")
                    augs(T0, 0)
                    augs(T1, 1)
                    mains(T0, 0)
                    post(T0, 0)
                    mains(T1, 1)
                    post(T1, 1)
                else:
                    for u in range(2):
                        # unit = row-groups (2u, 2u+1) x full K; 4 banks
                        T = psT.tile([128, 2, K], dt.float32, tag="T", name=f"T{u}")
                        augs(T, u)
                        mains(T, u)
                        post(T, u)
                if s % 4 == 3:
                    # combine the two accumulator halves on-chip; DEFER the
                    # store trigger one block so its wait-for-DVE is long
                    # satisfied when the sync queue reaches it (an inline
                    # store stalls all later xt prefetch triggers).
                    osbF = osbp.tile([128, 16], dt.float32, tag="osbF")
                    _tt_add(nc, mybir, osbF[:], osbA[:], osbB[:])
                    if pending_store is not None:
                        nc.sync.dma_start(*pending_store)
                    pending_store = (
                        out_d[(s - 3) * F : (s + 1) * F].rearrange(
                            "(p q) -> p q", p=128
                        ),
                        osbF[:],
                    )
            nc.sync.dma_start(*pending_store)
    nc.compile()
    return nc


def _pack_pairs(a):
    """[D, M] -> [2, 128, 2, M] with d = 256*c + 128*e + p (DoubleRow pairs)."""
    d, m = a.shape
    return np.ascontiguousarray(a.reshape(2, 2, 128, m).transpose(0, 2, 1, 3))


def _host_prep_shared(center, var, pr, threshold):
    import concourse.mybir as mybir

    fp8 = mybir.dt.np(mybir.dt.float8e4)
    C64 = center.astype(np.float64)  # [K, D]
    w = pr.astype(np.float64) * var.astype(np.float64)  # [K]
    # permute columns: the KS smallest-w go last. For the rest, s_k = 1/w_k
    # keeps the fp8 cm in range because w is bounded below by the KS-th
    # order statistic (~0.066 for this distribution).
    order = np.argsort(w, kind="stable")
    perm = np.concatenate([np.sort(order[KS:]), np.sort(order[:KS])])
    Cp = C64[perm]
    wp = w[perm]
    s = np.empty(K)
    s[:KB] = 1.0 / wp[:KB]
    s[KB:] = S_SMALL
    cmF = np.ascontiguousarray((-2.0 * Cp * s[:, None]).T)  # [D, K]
    assert np.abs(cmF).max() < 432.0, np.abs(cmF).max()
    cmT = cmF.astype(fp8)
    cm = _pack_pairs(cmT)
    # consistent s*csq from the rounded cm: the effective center is
    # c_hat = -cm/(2 s), so s*||c_hat||^2 = sum_d cm^2 / (4 s)
    cmf = cmT.astype(np.float64)
    cs = ((cmf**2).sum(0) / (4.0 * s)).astype(np.float32)
    cs_hi = cs.astype(BF16)
    cs_lo = (cs - cs_hi.astype(np.float32)).astype(BF16)
    s32 = s.astype(np.float32)
    s_hi = s32.astype(BF16)
    s_lo = (s32 - s_hi.astype(np.float32)).astype(BF16)
    # rhs rows pair with lhsT rows [xsq_hi, xsq_hi, xsq_lo, 1, 1]:
    # s*xsq via the 3-term hi/lo product, s*csq via 2 terms.
    aug_rows = np.stack([s_hi, s_lo, s_hi, cs_hi, cs_lo])  # [5, K]
    # DVE correction multiplier for the small tail: the ACT accum already
    # counted (1/16)/sqdist for them, true weight is w_k/sqdist, and the
    # dump holds rr = 1/(16*sqdist) -> multiplier = 16*w_k - 1.
    wks = np.ascontiguousarray(
        (wp[KB:] * S_SMALL - 1.0).astype(np.float32).astype(BF16)[None, :]
    )
    return cm, aug_rows, wks


def _host_prep_shard(Xs, aug_rows):
    import concourse.mybir as mybir

    fp8 = mybir.dt.np(mybir.dt.float8e4)
    Xq = Xs.astype(fp8)
    xtT = np.ascontiguousarray(Xq.T)  # [D, R]
    xt = _pack_pairs(xtT)
    xsq = (Xq.astype(np.float32) ** 2).sum(1, dtype=np.float64).astype(np.float32)
    xsq_hi = xsq.astype(BF16)
    xsq_lo = (xsq - xsq_hi.astype(np.float32)).astype(BF16)
    onesr = np.ones(Xs.shape[0], BF16)
    arx = np.stack([xsq_hi, xsq_hi, xsq_lo, onesr, onesr])
    # compact const: [AUGN, 2*KH + R] = aug rhs rows ++ raw arx columns,
    # replicated 4x on the host so the two queues fill partition groups
    # 0/32/64/96 fast; group g slices arx columns s*512+128g..+128 as its
    # lhsT.
    cq = np.concatenate([aug_rows.astype(BF16), arx.astype(BF16)], axis=1)
    cq4 = np.broadcast_to(cq[None], (4,) + cq.shape)
    return xt, np.ascontiguousarray(cq4)


def kernel(X, center, var, pr, threshold):
    global _NC
    X = np.asarray(X)
    cm, aug_rows, wks = _host_prep_shared(
        np.asarray(center), np.asarray(var), np.asarray(pr), np.asarray(threshold)
    )
    in_maps = []
    for c in range(NCORES):
        xt, cq = _host_prep_shard(X[c * R : (c + 1) * R], aug_rows)
        in_maps.append(dict(xt=xt, cq=cq, cm=cm, wk=wks))

    if _NC is None:
        _NC = _build_nc()

    from concourse.bass_utils import run_bass_kernel_spmd

    res = run_bass_kernel_spmd(_NC, in_maps, core_ids=list(range(NCORES)))
    parts = []
    for c in range(NCORES):
        y = res.results[c]["out"].reshape(NSUP // 4, 128, 4, 4)  # [s4, p, sl, a]
        parts.append(y.transpose(0, 2, 3, 1).reshape(R))  # [s4, sl, a, p]
    out = np.concatenate(parts)
    thv = np.float32(np.asarray(threshold).reshape(-1)[0])
    return np.ascontiguousarray(out - thv, dtype=np.float32)


# revision 31
# speedup vs baseline: 1.0319x; 1.0319x over previous
"""Trainium2 Bass kernel for nn_DetectorKmeans (retrieval_knn).

density[n] = sum_k (pr[k]*var[k]) / ||X[n]-C[k]||^2  - threshold

Data-parallel over 8 NeuronCores (X sharded along N). Per core, per
"unit" = (256-row half-supertile, full K):

  * The per-column weight w_k is FOLDED INTO THE MATMUL via a per-column
    scale s_k: PSUM T[:,k] = s_k * sqdist. For the 768 largest-w columns
    s_k = 1/w_k, so ACT's Reciprocal emits w_k/sqdist directly and its
    free-dim accum_out produces the weighted k-sum AT FP32 FOR FREE.
    The 256 smallest-w columns (where 1/w_k would overflow the fp8 cm)
    use s_k = 16 and are host-permuted to k-positions 768..1023; a DVE
    reciprocal + narrow scalar_tensor_tensor (x 16*w_k) covers them.
    This removes the full-width DVE reduce (was 2 x 1024-elem STT at a
    fixed 2 cycles/elem = the kernel-wide bottleneck tail).
  * PSUM buffer [128, 2, 1024] (4 banks; pool bufs=2 = all 8 banks).
  * 5-row augmented matmuls run in disjoint 32-row PE groups
    (tile_position=(32t,0)), adding s_k*xsq[n] (3-term bf16 hi/lo
    product) and s_k*csq[k] (2-term) for the 4 row-tiles; then fp8
    DoubleRow mains (2 contraction chunks of 256) accumulate the cross
    term at 2x bf16 streaming rate.
  * DMA queues: sync = cq (aug const, host-replicated 4x) + xt stream +
    output stores; scalar = ACT table load, then cm (h=0 halves first,
    matching the h-outer main order) + wk_small.
"""

import numpy as np
import ml_dtypes

BF16 = ml_dtypes.bfloat16

N, K, D = 65536, 1024, 512
NCORES = 8
R = N // NCORES
F = 512  # rows per supertile
KH = 512  # k-half (PSUM bank width in fp32)
NSUP = R // F
KS = 128  # small-w columns handled by DVE (host-permuted to the tail)
KB = K - KS
S_SMALL = 16.0  # power of two: exact fp8 exponent shift
AUGN = 5

_NC = None


def _act_recip(nc, mybir, out, in_, accum_out=None):
    """ACT-engine reciprocal (bypasses the library guard; measured max rel
    err ~1.2e-5 on TRN2 HW for this kernel's value range). With accum_out
    the engine also emits the free-dim sum at fp32 -- the weighted reduce
    comes for free because w is pre-folded into the PSUM column scale."""
    dt = mybir.dt
    eng = nc.scalar
    ins = [
        eng.lower_ap(in_),
        mybir.ImmediateValue(dtype=dt.float32, value=0.0),
        mybir.ImmediateValue(dtype=dt.float32, value=1.0),
        mybir.ImmediateValue(dtype=dt.float32, value=0.0),
    ]
    outs = [eng.lower_ap(out)]
    if accum_out is not None:
        outs.append(eng.lower_ap(accum_out))
    return eng.add_instruction(
        mybir.InstActivation(
            name=nc.get_next_instruction_name(),
            func=mybir.ActivationFunctionType.Reciprocal,
            ins=ins,
            outs=outs,
        )
    )


def _tt_add(nc, mybir, out, in0, in1):
    """DVE tensor_tensor add (no bass builder exists)."""
    eng = nc.vector
    return eng.add_instruction(
        mybir.InstTensorTensor(
            name=nc.get_next_instruction_name(),
            op=mybir.AluOpType.add,
            ins=[eng.lower_ap(in0), eng.lower_ap(in1)],
            outs=[eng.lower_ap(out)],
        )
    )


def _build_nc(r=R, num_devices=NCORES):
    import concourse.bacc as bacc
    import concourse.tile as tile
    import concourse.mybir as mybir

    import os

    dt = mybir.dt
    nsup = r // F
    cqw = 2 * KH + r
    nc = bacc.Bacc(
        "TRN2", target_bir_lowering=False, debug=False, num_devices=num_devices
    )
    _salt = os.environ.get("KERNEL_SALT", "")
    xt_d = nc.dram_tensor("xt", [2, 128, 2, r], dt.float8e4, kind="ExternalInput")
    cm_d = nc.dram_tensor("cm", [2, 128, 2, K], dt.float8e4, kind="ExternalInput")
    wk_d = nc.dram_tensor("wk", [1, KS], dt.bfloat16, kind="ExternalInput")
    cq_d = nc.dram_tensor("cq", [4, AUGN, cqw], dt.bfloat16, kind="ExternalInput")
    out_d = nc.dram_tensor("out", [r], dt.float32, kind="ExternalOutput")

    with tile.TileContext(nc) as tc:
        with (
            tc.tile_pool(name="const" + _salt, bufs=1) as constp,
            tc.tile_pool(name="xin", bufs=4) as xinp,
            tc.tile_pool(name="rec", bufs=4) as recp,
            tc.tile_pool(name="osb", bufs=4) as osbp,
            tc.tile_pool(name="psT", bufs=2, space="PSUM") as psT,
        ):
            # cq groups 0/1 on sync, 2/3 on scalar -- two queues drain the
            # 4 small triggers in parallel; host replicated the rows 4x.
            cq = constp.tile([128, cqw], dt.bfloat16)
            for g in range(2):
                nc.sync.dma_start(cq[32 * g : 32 * g + AUGN, :], cq_d[g])
            for g in range(2, 4):
                nc.scalar.dma_start(cq[32 * g : 32 * g + AUGN, :], cq_d[g])
            carq = cq[:, : 2 * KH].rearrange("p (h k) -> p h k", h=2)
            auga = cq[:, 2 * KH :]
            # cm on the scalar queue, h=0 halves first (matches h-outer
            # main order so unit 0 h=0 can start earliest).
            cm = constp.tile([128, 2, 2, K], dt.float8e4)
            cm_r = cm_d.rearrange("c p e k -> p c e k")
            for h in range(2):
                for c in range(2):
                    nc.scalar.dma_start(
                        cm[:, c, :, KH * h : KH * (h + 1)],
                        cm_r[:, c, :, KH * h : KH * (h + 1)],
                    )
            wks = constp.tile([128, KS], dt.bfloat16)
            nc.scalar.dma_start(wks[:], wk_d[:].partition_broadcast(128))
            xt_r = xt_d.rearrange("c p e n -> p c e n")

            pending_store = None
            for s in range(nsup):
                n0 = s * F
                xt = xinp.tile([128, 2, 2, F], dt.float8e4, tag="xt")
                for c in range(2):
                    nc.sync.dma_start(xt[:, c, :, :], xt_r[:, c, :, n0 : n0 + F])
                if s % 4 == 0:
                    osbA = osbp.tile([128, 16], dt.float32, tag="osbA")
                    osbB = osbp.tile([128, 16], dt.float32, tag="osbB")

                def augs(T, u):
                    # all four aug matmuls in disjoint row groups -> one
                    # concurrent span; every partition group holds ALL arx
                    # columns, so the h=1 augs just read group g's columns
                    # from row group g+2.
                    for h in range(2):
                        for tl in range(2):
                            g = 2 * u + tl
                            gp = g if h == 0 else (g + 2) % 4
                            a0 = n0 + 128 * g
                            nc.tensor.matmul(
                                T[:, tl, KH * h : KH * (h + 1)],
                                auga[32 * gp : 32 * gp + AUGN, a0 : a0 + 128],
                                carq[32 * gp : 32 * gp + AUGN, h, :],
                                start=True,
                                stop=False,
                                tile_position=(32 * gp, 0),
                            )

                def mains(T, u):
                    # h-outer so the first unit only waits on the h=0 half
                    # of cm; accumulation group per (tl, h) stays c0->c1.
                    for h in range(2):
                        for tl in range(2):
                            g = 2 * u + tl
                            for c in range(2):
                                nc.tensor.matmul(
                                    T[:, tl, KH * h : KH * (h + 1)],
                                    xt[:, c, :, 128 * g : 128 * (g + 1)],
                                    cm[:, c, :, KH * h : KH * (h + 1)],
                                    perf_mode=mybir.MatmulPerfMode.DoubleRow,
                                    start=False,
                                    stop=(c == 1),
                                )

                def post(T, u):
                    # w is folded into the PSUM column scale, so the row
                    # density is a PLAIN sum of the reciprocal dump.
                    # Balance that reduce across engines: 1/4 of units use
                    # ACT's free-dim accum (costs +130ns instr split +
                    # 288ns accumulator read each), the rest use 1-source
                    # DVE tensor_scalar sums (single read port -> the
                    # accumulator readback port stays free, full rate).
                    # Small tail columns (weighted 1/16) get a narrow STT
                    # correction of sum (16 w_k - 1) * rr either way.
                    scr = recp.tile([128, 2, KS], dt.bfloat16, tag="scr")
                    dump = recp.tile([128, 2, K], dt.bfloat16, tag="dump")
                    # 12 of 32 units use the ACT-accum path. Placement:
                    # sparse (1 in 4) early so ACT never sustains a rate
                    # above PE's unit period, denser (1 in 2, alternating)
                    # late so DVE's tensor_reduce backlog drains before the
                    # last matmul instead of trailing it.
                    gi = 2 * s + u
                    act_mode = (gi % 4 == 0) if gi < 16 else (gi % 2 == 1)
                    if act_mode:
                        for tl in range(2):
                            col = 4 * (s % 4) + 2 * u + tl
                            _act_recip(
                                nc,
                                mybir,
                                dump[:, tl, :],
                                T[:, tl, :],
                                accum_out=osbA[:, col : col + 1],
                            )
                    else:
                        _act_recip(nc, mybir, dump[:], T[:])
                        for tl in range(2):
                            col = 4 * (s % 4) + 2 * u + tl
                            nc.vector.tensor_reduce(
                                osbA[:, col : col + 1],
                                dump[:, tl, :],
                                axis=mybir.AxisListType.X,
                                op=mybir.AluOpType.add,
                            )
                    for tl in range(2):
                        col = 4 * (s % 4) + 2 * u + tl
                        nc.vector.scalar_tensor_tensor(
                            scr[:, tl, :],
                            dump[:, tl, KB:],
                            0.0,
                            wks[:],
                            op0=mybir.AluOpType.bypass,
                            op1=mybir.AluOpType.mult,
                            accum_out=osbB[:, col : col + 1],
                        )

                if s == 0:
                    # pipeline fill: both units' augs run as soon as cq
                    # lands (each aug's weight load waits only on its own
                    # cq group's DMA -- verified minimal in the BIR).
                    T0 = psT.tile([128, 2, K], dt.float32, tag="T", name="T0")
                    T1 = psT.tile([128, 2, K], dt.float32, tag="T", name="T1")
                    augs(T0, 0)
                    augs(T1, 1)
                    mains(T0, 0)
                    post(T0, 0)
                    mains(T1, 1)
                    post(T1, 1)
                else:
                    for u in range(2):
                        # unit = row-groups (2u, 2u+1) x full K; 4 banks
                        T = psT.tile([128, 2, K], dt.float32, tag="T", name=f"T{u}")
                        augs(T, u)
                        mains(T, u)
                        post(T, u)
                if s % 4 == 3:
                    # combine the two accumulator halves on-chip; DEFER the
                    # store trigger one block so its wait-for-DVE is long
                    # satisfied when the sync queue reaches it (an inline
                    # store stalls all later xt prefetch triggers).
                    osbF = osbp.tile([128, 16], dt.float32, tag="osbF")
                    _tt_add(nc, mybir, osbF[:], osbA[:], osbB[:])
                    if pending_store is not None:
                        nc.sync.dma_start(*pending_store)
                    pending_store = (
                        out_d[(s - 3) * F : (s + 1) * F].rearrange(
                            "(p q) -> p q", p=128
                        ),
                        osbF[:],
                    )
            nc.sync.dma_start(*pending_store)
    nc.compile()
    return nc


def _pack_pairs(a):
    """[D, M] -> [2, 128, 2, M] with d = 256*c + 128*e + p (DoubleRow pairs)."""
    d, m = a.shape
    return np.ascontiguousarray(a.reshape(2, 2, 128, m).transpose(0, 2, 1, 3))


def _host_prep_shared(center, var, pr, threshold):
    import concourse.mybir as mybir

    fp8 = mybir.dt.np(mybir.dt.float8e4)
    C64 = center.astype(np.float64)  # [K, D]
    w = pr.astype(np.float64) * var.astype(np.float64)  # [K]
    # permute columns: the KS smallest-w go last. For the rest, s_k = 1/w_k
    # keeps the fp8 cm in range because w is bounded below by the KS-th
    # order statistic (~0.066 for this distribution).
    order = np.argsort(w, kind="stable")
    perm = np.concatenate([np.sort(order[KS:]), np.sort(order[:KS])])
    Cp = C64[perm]
    wp = w[perm]
    s = np.empty(K)
    s[:KB] = 1.0 / wp[:KB]
    s[KB:] = S_SMALL
    cmF = np.ascontiguousarray((-2.0 * Cp * s[:, None]).T)  # [D, K]
    assert np.abs(cmF).max() < 432.0, np.abs(cmF).max()
    cmT = cmF.astype(fp8)
    cm = _pack_pairs(cmT)
    # consistent s*csq from the rounded cm: the effective center is
    # c_hat = -cm/(2 s), so s*||c_hat||^2 = sum_d cm^2 / (4 s)
    cmf = cmT.astype(np.float64)
    cs = ((cmf**2).sum(0) / (4.0 * s)).astype(np.float32)
    cs_hi = cs.astype(BF16)
    cs_lo = (cs - cs_hi.astype(np.float32)).astype(BF16)
    s32 = s.astype(np.float32)
    s_hi = s32.astype(BF16)
    s_lo = (s32 - s_hi.astype(np.float32)).astype(BF16)
    # rhs rows pair with lhsT rows [xsq_hi, xsq_hi, xsq_lo, 1, 1]:
    # s*xsq via the 3-term hi/lo product, s*csq via 2 terms.
    aug_rows = np.stack([s_hi, s_lo, s_hi, cs_hi, cs_lo])  # [5, K]
    # DVE correction multiplier for the small tail: the ACT accum already
    # counted (1/16)/sqdist for them, true weight is w_k/sqdist, and the
    # dump holds rr = 1/(16*sqdist) -> multiplier = 16*w_k - 1.
    wks = np.ascontiguousarray(
        (wp[KB:] * S_SMALL - 1.0).astype(np.float32).astype(BF16)[None, :]
    )
    return cm, aug_rows, wks


def _host_prep_shard(Xs, aug_rows):
    import concourse.mybir as mybir

    fp8 = mybir.dt.np(mybir.dt.float8e4)
    Xq = Xs.astype(fp8)
    xtT = np.ascontiguousarray(Xq.T)  # [D, R]
    xt = _pack_pairs(xtT)
    xsq = (Xq.astype(np.float32) ** 2).sum(1, dtype=np.float64).astype(np.float32)
    xsq_hi = xsq.astype(BF16)
    xsq_lo = (xsq - xsq_hi.astype(np.float32)).astype(BF16)
    onesr = np.ones(Xs.shape[0], BF16)
    arx = np.stack([xsq_hi, xsq_hi, xsq_lo, onesr, onesr])
    # compact const: [AUGN, 2*KH + R] = aug rhs rows ++ raw arx columns,
    # replicated 4x on the host so the two queues fill partition groups
    # 0/32/64/96 fast; group g slices arx columns s*512+128g..+128 as its
    # lhsT.
    cq = np.concatenate([aug_rows.astype(BF16), arx.astype(BF16)], axis=1)
    cq4 = np.broadcast_to(cq[None], (4,) + cq.shape)
    return xt, np.ascontiguousarray(cq4)


def kernel(X, center, var, pr, threshold):
    global _NC
    X = np.asarray(X)
    cm, aug_rows, wks = _host_prep_shared(
        np.asarray(center), np.asarray(var), np.asarray(pr), np.asarray(threshold)
    )
    in_maps = []
    for c in range(NCORES):
        xt, cq = _host_prep_shard(X[c * R : (c + 1) * R], aug_rows)
        in_maps.append(dict(xt=xt, cq=cq, cm=cm, wk=wks))

    if _NC is None:
        _NC = _build_nc()

    from concourse.bass_utils import run_bass_kernel_spmd

    res = run_bass_kernel_spmd(_NC, in_maps, core_ids=list(range(NCORES)))
    parts = []
    for c in range(NCORES):
        y = res.results[c]["out"].reshape(NSUP // 4, 128, 4, 4)  # [s4, p, sl, a]
        parts.append(y.transpose(0, 2, 3, 1).reshape(R))  # [s4, sl, a, p]
    out = np.concatenate(parts)
    thv = np.float32(np.asarray(threshold).reshape(-1)[0])
    return np.ascontiguousarray(out - thv, dtype=np.float32)
